# revision 1
# baseline (speedup 1.0000x reference)
"""Trainium2 Bass kernel for nn_AttentionLayer_88399016887055.

Math (per head h, B=1):
  w = W[h] @ ft + b[h]                               # [N]
  s_ij = leaky_relu(w_i + w_j, 0.2) + bias[i, j]
  a = softmax_rows(s)                                # [N, N]
  t[h] = a @ a^T                                     # [N, N]
  out[:, i, h*N + k] = t[h][i, k]

Kernel formulation (per core):
  Work in the transposed matrix C[j, i] = exp(x[i, j] - SHIFT) where
  x[i, j] = lrelu(w_i + w_j) + bias[i, j].  With column sums
  s_i = sum_j C[j, i]:  t[i, k] = (C^T C)[i, k] / (s_i * s_k).
  The PE contracts over the partition axis, so C strips [128 j, N i]
  feed matmuls directly with no on-chip transpose; bias arrives
  pre-transposed from the host.  No row-max subtraction is needed
  (logits are bounded, exp stays well inside fp32/fp16 range).

Sharding: 8 cores = (head h, row-half off in {0, 2048}).  The "off"
cores compute in rolled coordinates (ft and bias rolled by -off along
both N axes on the host), so a single SPMD program computes rows
[0, 2048) for every core; the host un-rolls the output columns.

Per-core schedule (j split into two phases of 16 strips so the fp16 C
half [16 MB] stays SBUF-resident):
  phase A: elementwise C strips 0..15, column sums, G_partial = C_A^T C_A
           spilled to DRAM scratch.
  phase B: elementwise C strips 16..31, sums, total reciprocal, then
           t = (G_partial + C_B^T C_B) * r_i * r_k streamed to out.
"""

import sys

import numpy as np

sys.path.insert(0, "/opt/trn_rl_repo")

import concourse.bass as bass
import concourse.bacc as bacc
import concourse.mybir as mybir
import concourse.tile as tile
from concourse.bass_utils import run_bass_kernel_spmd
from concourse.tile_rust import add_dep_helper

F32 = mybir.dt.float32
F16 = mybir.dt.float16
AF = mybir.ActivationFunctionType

N = 4096
FT = 512
H = 4
NCORES = 8
SHIFT = 2.7725887  # 4*ln(2): keeps exp() comfortably inside fp16 range
ALPHA = 0.2
# The HW Lrelu LUT has a fixed negative slope SIGMA=0.01 (alpha is ignored).
# lrelu_0.2(z) == LA*f(z) + LB*f(-z) for f = lrelu_sigma: solve
#   a - sigma*b = 1,  sigma*a - b = 0.2
SIGMA = 0.01
LB = (ALPHA - SIGMA) / (SIGMA * SIGMA - 1.0)
LA = 1.0 + SIGMA * LB
ALU = mybir.AluOpType


def build_kernel(n=N, ft_dim=FT, upto=99):
    ns = n // 128            # total j strips
    nh = ns // 2             # strips per phase
    half = n // 2            # output rows per core
    qw = min(1024, n)        # elementwise sub-tile width
    kw = min(1024, n)        # matmul psum tile k-width (2 PSUM banks)
    cw = min(512, n)         # row-chunk width for [1, *] psum evictions
    nf = ft_dim // 128

    nc = bacc.Bacc(None, target_bir_lowering=False, debug=False)
    ftr = nc.dram_tensor("ftr", [ft_dim, n], F32, kind="ExternalInput")
    biasT = nc.dram_tensor("biasT", [n, n], F32, kind="ExternalInput")
    wh = nc.dram_tensor("wh", [1, ft_dim], F32, kind="ExternalInput")
    bh = nc.dram_tensor("bh", [1, 1], F32, kind="ExternalInput")
    out = nc.dram_tensor("out", [half, n], F32, kind="ExternalOutput")
    w_scr = nc.dram_tensor("w_scr", [1, n], F32)
    sa_scr = nc.dram_tensor("sa_scr", [1, n], F32)
    sb_scr = nc.dram_tensor("sb_scr", [1, n], F32)
    r_scr = nc.dram_tensor("r_scr", [1, n], F32)
    g_scr = nc.dram_tensor("g_scr", [half, n], F32)

    with tile.TileContext(nc) as tc:
        with tc.tile_pool(name="persist", bufs=1) as P, \
                tc.tile_pool(name="work", bufs=1) as WK, \
                tc.tile_pool(name="pwork", bufs=1, space="PSUM") as PW:
            ones_c = P.tile([128, 1], F16, tag="ones_c")
            negshift = P.tile([128, 1], F32, tag="negshift")
            whT = P.tile([128, nf], F32, tag="whT")
            bhs = P.tile([1, 1], F32, tag="bhs")
            wrow = P.tile([128, n], F32, tag="wrow")
            wcols = P.tile([128, ns], F32, tag="wcols")
            negwcols = P.tile([128, ns], F32, tag="negwcols")
            rcols = P.tile([128, ns], F32, tag="rcols")
            scolsa = P.tile([128, ns], F32, tag="scolsa")
            scolsb = P.tile([128, ns], F32, tag="scolsb")
            rbc = P.tile([128, n], F32, tag="rbc")
            C = [P.tile([128, n], F16, tag=f"C{s}", name=f"C{s}")
                 for s in range(nh)]

            nc.vector.memset(ones_c[:], 1.0)
            nc.vector.memset(negshift[:], -SHIFT)
            nc.sync.dma_start(whT[:], wh[0, :].rearrange("(f p) -> p f", p=128))
            nc.sync.dma_start(bhs[:], bh[:, :])

            # ---- w row: w = W[h] @ ft + b[h], evicted chunk-wise to DRAM ----
            w_stores = []
            for q in range(n // kw):
                fq = []
                for f in range(nf):
                    ftile = WK.tile([128, kw], F32, tag="ev", bufs=4, name=f"ft{q}_{f}")
                    nc.sync.dma_start(
                        ftile[:], ftr[f * 128:(f + 1) * 128, q * kw:(q + 1) * kw])
                    fq.append(ftile)
                for c in range(kw // cw):
                    psw = PW.tile([1, cw], F32, tag="prow", bufs=2, name=f"psw{q}_{c}")
                    for f in range(nf):
                        nc.tensor.matmul(
                            psw[:], whT[:, f:f + 1],
                            fq[f][:, c * cw:(c + 1) * cw],
                            start=(f == 0), stop=(f == nf - 1))
                    wev = WK.tile([1, cw], F32, tag="wev", bufs=2, name=f"wev{q}_{c}")
                    nc.scalar.activation(wev[:], psw[:], AF.Identity, bias=bhs[:])
                    w_stores.append(nc.sync.dma_start(
                        w_scr[0:1, q * kw + c * cw:q * kw + (c + 1) * cw], wev[:]))

            ld_wc = nc.sync.dma_start(
                wcols[:], w_scr[0, :].rearrange("(t p) -> p t", p=128))
            ld_wr = nc.sync.dma_start(
                wrow[:], w_scr[0, :][None, :].to_broadcast((128, n)))
            for st in w_stores:
                add_dep_helper(ld_wc.ins, st.ins, reason="w_scr RAW cols")
                add_dep_helper(ld_wr.ins, st.ins, reason="w_scr RAW row")
            nc.vector.tensor_scalar_mul(negwcols[:], wcols[:], -1.0)

            g_store = {}
            s_stores = [[], []]

            for ph in range(2 if upto >= 2 else 0):
                s_scr = sa_scr if ph == 0 else sb_scr
                # ---- elementwise: C[j, i] strips for this j half ----
                for s in range(nh):
                    js = ph * nh + s
                    for q in range(n // qw):
                        lo = q * qw
                        bt = WK.tile([128, qw], F32, tag="bt",
                                     bufs=2, name=f"bt{ph}_{s}_{q}")
                        nc.sync.dma_start(
                            bt[:], biasT[js * 128:(js + 1) * 128, lo:lo + qw])
                        sc = WK.tile([128, qw], F32, tag="sc",
                                     bufs=2, name=f"sc{ph}_{s}_{q}")
                        nc.scalar.activation(
                            sc[:], wrow[:, lo:lo + qw], AF.Lrelu,
                            bias=wcols[:, js:js + 1])
                        sc2 = WK.tile([128, qw], F32, tag="sc2",
                                      bufs=2, name=f"sc2{ph}_{s}_{q}")
                        nc.scalar.activation(
                            sc2[:], wrow[:, lo:lo + qw], AF.Lrelu,
                            bias=negwcols[:, js:js + 1], scale=-1.0)
                        nc.vector.scalar_tensor_tensor(
                            sc2[:], sc2[:], LB, bt[:], op0=ALU.mult, op1=ALU.add)
                        nc.vector.scalar_tensor_tensor(
                            sc[:], sc[:], LA, sc2[:], op0=ALU.mult, op1=ALU.add)
                        nc.scalar.activation(
                            C[s][:, lo:lo + qw], sc[:], AF.Exp,
                            bias=negshift[:])

                # ---- column sums of this half (PE contracts partitions) ----
                for c in range(n // cw if upto >= 3 else 0):
                    pss = PW.tile([1, cw], F32, tag="prow", bufs=2, name=f"pss{ph}_{c}")
                    for s in range(nh):
                        nc.tensor.matmul(
                            pss[:], ones_c[:], C[s][:, c * cw:(c + 1) * cw],
                            start=(s == 0), stop=(s == nh - 1))
                    sev = WK.tile([1, cw], F32, tag="wev", bufs=2,
                                   name=f"sev{ph}_{c}")
                    nc.scalar.copy(sev[:], pss[:])
                    s_stores[ph].append(nc.sync.dma_start(
                        s_scr[0:1, c * cw:(c + 1) * cw], sev[:]))

                if ph == 1 and upto >= 4:
                    # total sums -> reciprocals (columns), r broadcast (rows)
                    ld_sa = nc.sync.dma_start(
                        scolsa[:], sa_scr[0, :].rearrange("(t p) -> p t", p=128))
                    ld_sb = nc.sync.dma_start(
                        scolsb[:], sb_scr[0, :].rearrange("(t p) -> p t", p=128))
                    for st in s_stores[0]:
                        add_dep_helper(ld_sa.ins, st.ins, reason="sa RAW")
                    for st in s_stores[1]:
                        add_dep_helper(ld_sb.ins, st.ins, reason="sb RAW")
                    nc.vector.tensor_add(scolsa[:], scolsa[:], scolsb[:])
                    nc.vector.reciprocal(rcols[:], scolsa[:])
                    st_r = nc.sync.dma_start(
                        r_scr[0, :].rearrange("(t p) -> p t", p=128), rcols[:])
                    ld_rb = nc.sync.dma_start(
                        rbc[:], r_scr[0, :][None, :].to_broadcast((128, n)))
                    add_dep_helper(ld_rb.ins, st_r.ins, reason="r_scr RAW")

                # ---- G accumulation: ph0 spills partials, ph1 finalizes ----
                for ib in range(half // 128 if upto >= 5 else 0):
                    for kh in range(n // kw):
                        ps = PW.tile([128, kw], F32, tag="ps",
                                     bufs=3, name=f"ps{ph}_{ib}_{kh}")
                        for s in range(nh):
                            for cc in range(kw // 512):
                                col = kh * kw + cc * 512
                                nc.tensor.matmul(
                                    ps[:, cc * 512:(cc + 1) * 512],
                                    C[s][:, ib * 128:(ib + 1) * 128],
                                    C[s][:, col:col + 512],
                                    start=(s == 0), stop=(s == nh - 1))
                        if ph == 0:
                            ev = WK.tile([128, kw], F32, tag="ev",
                                         bufs=6, name=f"ev{ib}_{kh}")
                            nc.scalar.copy(ev[:], ps[:])
                            g_store[(ib, kh)] = nc.sync.dma_start(
                                g_scr[ib * 128:(ib + 1) * 128,
                                      kh * kw:(kh + 1) * kw], ev[:])
                        else:
                            ga = WK.tile([128, kw], F32, tag="ev",
                                         bufs=6, name=f"ga{ib}_{kh}")
                            ld = nc.sync.dma_start(
                                ga[:], g_scr[ib * 128:(ib + 1) * 128,
                                             kh * kw:(kh + 1) * kw])
                            add_dep_helper(
                                ld.ins, g_store[(ib, kh)].ins, reason="g RAW")
                            nc.vector.tensor_add(ga[:], ps[:], ga[:])
                            nc.scalar.activation(
                                ga[:], ga[:], AF.Copy,
                                scale=rcols[:, ib:ib + 1])
                            nc.vector.tensor_mul(
                                ga[:], ga[:], rbc[:, kh * kw:(kh + 1) * kw])
                            nc.sync.dma_start(
                                out[ib * 128:(ib + 1) * 128,
                                    kh * kw:(kh + 1) * kw], ga[:])
    return nc


def make_core_inputs(ft_mat, bias_mat, W, b, n=N):
    """Host-side shard prep: one input map per core (head, roll-offset)."""
    half = n // 2
    ft0 = np.asarray(ft_mat, dtype=np.float32)[0]
    bias0 = np.asarray(bias_mat, dtype=np.float32)[0]
    biasT0 = np.ascontiguousarray(bias0.T)
    ins = []
    for core in range(NCORES):
        h = core % H
        off = (core // H) * half
        ftr = np.ascontiguousarray(np.roll(ft0, -off, axis=1))
        if off:
            bT = np.ascontiguousarray(np.roll(biasT0, -off, axis=(0, 1)))
        else:
            bT = biasT0
        ins.append({
            "ftr": ftr,
            "biasT": bT,
            "wh": np.ascontiguousarray(np.asarray(W, np.float32)[h]).reshape(1, -1),
            "bh": np.asarray(b, np.float32)[h].reshape(1, 1),
        })
    return ins


def assemble_output(results, n=N):
    half = n // 2
    full = np.zeros((1, n, H * n), dtype=np.float32)
    for core in range(NCORES):
        h = core % H
        off = (core // H) * half
        o = results[core]["out"]
        full[0, off:off + half, h * n:(h + 1) * n] = np.roll(o, off, axis=1)
    return full


_nc_cache = {}
USE_V2 = True


def kernel(ft_mat, bias_mat, W, b):
    key = (N, USE_V2)
    if key not in _nc_cache:
        nc = build_kernel_v2() if USE_V2 else build_kernel()
        nc.finalize()
        _nc_cache[key] = nc
    nc = _nc_cache[key]
    if USE_V2:
        ins = make_core_inputs_v2(ft_mat, bias_mat, W, b)
    else:
        ins = make_core_inputs(ft_mat, bias_mat, W, b)
    res = run_bass_kernel_spmd(nc, ins, list(range(NCORES)))
    return (assemble_output_v2 if USE_V2 else assemble_output)(res.results)


def build_kernel_v2(n=N, ft_dim=FT):
    """V2: exploit symmetry of t = C^T C.  Each core computes cyclic
    diagonals d in [0, nb/2] of the block grid for rows ib in [0, nb/2)
    and mirrors d in [1, nb/2) by PE-transposing finalized blocks.  The
    roll-offset core covers rows [nb/2, nb) of the same diagonal set, so
    together they tile the full symmetric matrix with ~half the MACs."""
    ns = n // 128
    nh = ns // 2
    half = n // 2
    nb = n // 128
    nbh = nb // 2            # ib range and also dmax
    dmax = nb // 2
    ngrp = (dmax + 3) // 4   # diagonal groups of up to 4 blocks (512 cols)
    qw = min(1024, n)
    cw = min(512, n)
    nf = ft_dim // 128
    gcols = dmax * 128 + 128

    nc = bacc.Bacc(None, target_bir_lowering=False, debug=False)
    ftr = nc.dram_tensor("ftr", [ft_dim, n], F32, kind="ExternalInput")
    biasT = nc.dram_tensor("biasT", [n, n], F32, kind="ExternalInput")
    wh = nc.dram_tensor("wh", [1, ft_dim], F32, kind="ExternalInput")
    bh = nc.dram_tensor("bh", [1, 1], F32, kind="ExternalInput")
    ident_d = nc.dram_tensor("ident", [128, 128], F32, kind="ExternalInput")
    out = nc.dram_tensor("out", [n, n], F32, kind="ExternalOutput")
    w_scr = nc.dram_tensor("w_scr", [1, n], F32)
    sa_scr = nc.dram_tensor("sa_scr", [1, n], F32)
    sb_scr = nc.dram_tensor("sb_scr", [1, n], F32)
    r_scr = nc.dram_tensor("r_scr", [1, n], F32)
    g_scr = nc.dram_tensor("g_scr", [half, gcols], F32)

    with tile.TileContext(nc) as tc:
        with tc.tile_pool(name="persist", bufs=1) as P, \
                tc.tile_pool(name="work", bufs=1) as WK, \
                tc.tile_pool(name="pwork", bufs=1, space="PSUM") as PW:
            ones_c = P.tile([128, 1], F16, tag="ones_c")
            negshift = P.tile([128, 1], F32, tag="negshift")
            whT = P.tile([128, nf], F32, tag="whT")
            bhs = P.tile([1, 1], F32, tag="bhs")
            ident = P.tile([128, 128], F32, tag="ident")
            wrow = P.tile([128, n], F32, tag="wrow")
            wcols = P.tile([128, ns], F32, tag="wcols")
            negwcols = P.tile([128, ns], F32, tag="negwcols")
            rcols = P.tile([128, ns], F32, tag="rcols")
            scolsa = P.tile([128, ns], F32, tag="scolsa")
            scolsb = P.tile([128, ns], F32, tag="scolsb")
            rbc = P.tile([128, n], F32, tag="rbc")
            C = [P.tile([128, n], F16, tag=f"C{s}", name=f"C{s}")
                 for s in range(nh)]

            nc.vector.memset(ones_c[:], 1.0)
            nc.vector.memset(negshift[:], -SHIFT)
            nc.sync.dma_start(whT[:], wh[0, :].rearrange("(f p) -> p f", p=128))
            nc.sync.dma_start(bhs[:], bh[:, :])
            nc.sync.dma_start(ident[:], ident_d[:, :])

            # ---- w row ----
            w_stores = []
            for q in range(n // qw):
                fq = []
                for f in range(nf):
                    ftile = WK.tile([128, qw], F32, tag="bt", bufs=2,
                                    name=f"ft{q}_{f}")
                    nc.sync.dma_start(
                        ftile[:], ftr[f * 128:(f + 1) * 128, q * qw:(q + 1) * qw])
                    fq.append(ftile)
                for c in range(qw // cw):
                    psw = PW.tile([1, cw], F32, tag="ps", bufs=6,
                                  name=f"psw{q}_{c}")
                    for f in range(nf):
                        nc.tensor.matmul(
                            psw[:], whT[:, f:f + 1],
                            fq[f][:, c * cw:(c + 1) * cw],
                            start=(f == 0), stop=(f == nf - 1))
                    wev = WK.tile([1, cw], F32, tag="wev", bufs=2,
                                  name=f"wev{q}_{c}")
                    nc.scalar.activation(wev[:], psw[:], AF.Identity, bias=bhs[:])
                    w_stores.append(nc.sync.dma_start(
                        w_scr[0:1, q * qw + c * cw:q * qw + (c + 1) * cw], wev[:]))

            ld_wc = nc.sync.dma_start(
                wcols[:], w_scr[0, :].rearrange("(t p) -> p t", p=128))
            ld_wr = nc.sync.dma_start(
                wrow[:], w_scr[0, :][None, :].to_broadcast((128, n)))
            for st in w_stores:
                add_dep_helper(ld_wc.ins, st.ins, reason="w RAW cols")
                add_dep_helper(ld_wr.ins, st.ins, reason="w RAW row")
            nc.vector.tensor_scalar_mul(negwcols[:], wcols[:], -1.0)

            g_store = {}
            s_stores = [[], []]

            for ph in range(2):
                s_scr = sa_scr if ph == 0 else sb_scr
                # ---- elementwise strips ----
                for s in range(nh):
                    js = ph * nh + s
                    for q in range(n // qw):
                        lo = q * qw
                        bt = WK.tile([128, qw], F32, tag="bt",
                                     bufs=2, name=f"bt{ph}_{s}_{q}")
                        nc.sync.dma_start(
                            bt[:], biasT[js * 128:(js + 1) * 128, lo:lo + qw])
                        sc = WK.tile([128, qw], F32, tag="sc",
                                     bufs=2, name=f"sc{ph}_{s}_{q}")
                        nc.scalar.activation(
                            sc[:], wrow[:, lo:lo + qw], AF.Lrelu,
                            bias=wcols[:, js:js + 1])
                        sc2 = WK.tile([128, qw], F32, tag="sc2",
                                      bufs=2, name=f"sc2{ph}_{s}_{q}")
                        nc.scalar.activation(
                            sc2[:], wrow[:, lo:lo + qw], AF.Lrelu,
                            bias=negwcols[:, js:js + 1], scale=-1.0)
                        nc.vector.scalar_tensor_tensor(
                            sc2[:], sc2[:], LB, bt[:], op0=ALU.mult, op1=ALU.add)
                        nc.vector.scalar_tensor_tensor(
                            sc[:], sc[:], LA, sc2[:], op0=ALU.mult, op1=ALU.add)
                        nc.scalar.activation(
                            C[s][:, lo:lo + qw], sc[:], AF.Exp,
                            bias=negshift[:])

                # ---- column sums ----
                for c in range(n // cw):
                    pss = PW.tile([1, cw], F32, tag="ps", bufs=6,
                                  name=f"pss{ph}_{c}")
                    for s in range(nh):
                        nc.tensor.matmul(
                            pss[:], ones_c[:], C[s][:, c * cw:(c + 1) * cw],
                            start=(s == 0), stop=(s == nh - 1))
                    sev = WK.tile([1, cw], F32, tag="wev", bufs=2,
                                  name=f"sev{ph}_{c}")
                    nc.scalar.copy(sev[:], pss[:])
                    s_stores[ph].append(nc.sync.dma_start(
                        s_scr[0:1, c * cw:(c + 1) * cw], sev[:]))

                if ph == 1:
                    ld_sa = nc.sync.dma_start(
                        scolsa[:], sa_scr[0, :].rearrange("(t p) -> p t", p=128))
                    ld_sb = nc.sync.dma_start(
                        scolsb[:], sb_scr[0, :].rearrange("(t p) -> p t", p=128))
                    for st in s_stores[0]:
                        add_dep_helper(ld_sa.ins, st.ins, reason="sa RAW")
                    for st in s_stores[1]:
                        add_dep_helper(ld_sb.ins, st.ins, reason="sb RAW")
                    nc.vector.tensor_add(scolsa[:], scolsa[:], scolsb[:])
                    nc.vector.reciprocal(rcols[:], scolsa[:])
                    st_r = nc.sync.dma_start(
                        r_scr[0, :].rearrange("(t p) -> p t", p=128), rcols[:])
                    ld_rb = nc.sync.dma_start(
                        rbc[:], r_scr[0, :][None, :].to_broadcast((128, n)))
                    add_dep_helper(ld_rb.ins, st_r.ins, reason="r RAW")

                # ---- symmetric G: diagonals d in [0, dmax), grouped ----
                for ib in range(nbh):
                    tiles = []
                    gws = []
                    for dg in range(ngrp):
                        gwd = min(4, dmax - dg * 4) * 128
                        gws.append(gwd)
                        tiles.append(PW.tile([128, gwd], F32, tag="ps", bufs=6,
                                             name=f"ps{ph}_{ib}_{dg}"))
                    for s in range(nh):
                        lhsT = C[s][:, ib * 128:(ib + 1) * 128]
                        for dg in range(ngrp):
                            c0 = (ib * 128 + dg * 512) % n
                            w1 = min(gws[dg], n - c0)
                            nc.tensor.matmul(
                                tiles[dg][:, 0:w1], lhsT, C[s][:, c0:c0 + w1],
                                start=(s == 0), stop=(s == nh - 1))
                            if w1 < gws[dg]:
                                nc.tensor.matmul(
                                    tiles[dg][:, w1:gws[dg]], lhsT,
                                    C[s][:, 0:gws[dg] - w1],
                                    start=(s == 0), stop=(s == nh - 1))
                    for dg in range(ngrp):
                        gwd = gws[dg]
                        c0 = (ib * 128 + dg * 512) % n
                        w1 = min(gwd, n - c0)
                        gc = dg * 512
                        if ph == 0:
                            ev = WK.tile([128, gwd], F32, tag="ev", bufs=6,
                                         name=f"ev{ib}_{dg}")
                            nc.scalar.copy(ev[:], tiles[dg][:])
                            g_store[(ib, dg)] = nc.sync.dma_start(
                                g_scr[ib * 128:(ib + 1) * 128, gc:gc + gwd],
                                ev[:])
                        else:
                            ga = WK.tile([128, gwd], F32, tag="ev", bufs=6,
                                         name=f"ga{ib}_{dg}")
                            ld = nc.sync.dma_start(
                                ga[:], g_scr[ib * 128:(ib + 1) * 128,
                                             gc:gc + gwd])
                            add_dep_helper(ld.ins, g_store[(ib, dg)].ins,
                                           reason="g RAW")
                            nc.vector.tensor_add(ga[:], tiles[dg][:], ga[:])
                            nc.scalar.activation(
                                ga[:], ga[:], AF.Copy,
                                scale=rcols[:, ib:ib + 1])
                            nc.vector.tensor_mul(
                                ga[:, 0:w1], ga[:, 0:w1], rbc[:, c0:c0 + w1])
                            nc.sync.dma_start(
                                out[ib * 128:(ib + 1) * 128, c0:c0 + w1],
                                ga[:, 0:w1])
                            if w1 < gwd:
                                nc.vector.tensor_mul(
                                    ga[:, w1:gwd], ga[:, w1:gwd],
                                    rbc[:, 0:gwd - w1])
                                nc.sync.dma_start(
                                    out[ib * 128:(ib + 1) * 128, 0:gwd - w1],
                                    ga[:, w1:gwd])
                            for dd in range(gwd // 128):
                                d = dg * 4 + dd
                                if d == 0:
                                    continue
                                kb = (ib + d) % nb
                                pt = PW.tile([128, 128], F32, tag="pt", bufs=2,
                                             name=f"pt{ib}_{d}")
                                nc.tensor.transpose(
                                    pt[:], ga[:, dd * 128:(dd + 1) * 128],
                                    ident[:])
                                mv = WK.tile([128, 128], F32, tag="mv", bufs=3,
                                             name=f"mv{ib}_{d}")
                                nc.scalar.copy(mv[:], pt[:])
                                nc.sync.dma_start(
                                    out[kb * 128:(kb + 1) * 128,
                                        ib * 128:(ib + 1) * 128], mv[:])

                # ---- d = dmax pass (self-paired diagonal, direct only) ----
                for ib in range(nbh):
                    kb = ib + dmax
                    pd = PW.tile([128, 128], F32, tag="ps", bufs=6,
                                 name=f"pd{ph}_{ib}")
                    for s in range(nh):
                        nc.tensor.matmul(
                            pd[:], C[s][:, ib * 128:(ib + 1) * 128],
                            C[s][:, kb * 128:(kb + 1) * 128],
                            start=(s == 0), stop=(s == nh - 1))
                    if ph == 0:
                        ev = WK.tile([128, 128], F32, tag="ev", bufs=6,
                                     name=f"evd{ib}")
                        nc.scalar.copy(ev[:], pd[:])
                        g_store[(ib, "dmax")] = nc.sync.dma_start(
                            g_scr[ib * 128:(ib + 1) * 128,
                                  dmax * 128:dmax * 128 + 128], ev[:])
                    else:
                        ga = WK.tile([128, 128], F32, tag="ev", bufs=6,
                                     name=f"gad{ib}")
                        ld = nc.sync.dma_start(
                            ga[:], g_scr[ib * 128:(ib + 1) * 128,
                                         dmax * 128:dmax * 128 + 128])
                        add_dep_helper(ld.ins, g_store[(ib, "dmax")].ins,
                                       reason="gd RAW")
                        nc.vector.tensor_add(ga[:], pd[:], ga[:])
                        nc.scalar.activation(
                            ga[:], ga[:], AF.Copy, scale=rcols[:, ib:ib + 1])
                        nc.vector.tensor_mul(
                            ga[:], ga[:], rbc[:, kb * 128:(kb + 1) * 128])
                        nc.sync.dma_start(
                            out[ib * 128:(ib + 1) * 128,
                                kb * 128:(kb + 1) * 128], ga[:])
    return nc


def make_core_inputs_v2(ft_mat, bias_mat, W, b, n=N):
    ins = make_core_inputs(ft_mat, bias_mat, W, b, n)
    eye = np.ascontiguousarray(np.eye(128, dtype=np.float32))
    for m in ins:
        m["ident"] = eye
    return ins


def block_mask_v2(n=N):
    nb = n // 128
    dmax = nb // 2
    maskA = np.zeros((nb, nb), bool)
    for ib in range(nb // 2):
        for d in range(dmax + 1):
            kb = (ib + d) % nb
            maskA[ib, kb] = True
            if 1 <= d <= dmax - 1:
                maskA[kb, ib] = True
    maskB = np.roll(maskA, (nb // 2, nb // 2), (0, 1))
    assert not (maskA & maskB).any() and (maskA | maskB).all(), \
        "v2 block split is not a disjoint cover"
    return maskA


def assemble_output_v2(results, n=N):
    maskA = block_mask_v2(n)
    mask_big = np.repeat(np.repeat(maskA, 128, 0), 128, 1)
    full = np.zeros((1, n, H * n), np.float32)
    for h in range(H):
        tA = results[h]["out"]
        tB = np.roll(results[h + H]["out"], n // 2, axis=(0, 1))
        full[0, :, h * n:(h + 1) * n] = np.where(mask_big, tA, tB)
    return full



# revision 7
# speedup vs baseline: 1.6548x; 1.6548x over previous
"""Trainium2 Bass kernel for nn_AttentionLayer_88399016887055.

Math (per head h, B=1):
  w = W[h] @ ft + b[h]                               # [N]
  s_ij = leaky_relu(w_i + w_j, 0.2) + bias[i, j]
  a = softmax_rows(s)                                # [N, N]
  t[h] = a @ a^T                                     # [N, N]
  out[:, i, h*N + k] = t[h][i, k]

Kernel formulation (per core):
  Work in the transposed matrix C[j, i] = exp(x[i, j] - SHIFT) where
  x[i, j] = lrelu(w_i + w_j) + bias[i, j].  With column sums
  s_i = sum_j C[j, i]:  t[i, k] = (C^T C)[i, k] / (s_i * s_k).
  The PE contracts over the partition axis, so C strips [128 j, N i]
  feed matmuls directly with no on-chip transpose; bias arrives
  pre-transposed from the host.  No row-max subtraction is needed
  (logits are bounded, exp stays well inside fp32/fp16 range).

Sharding: 8 cores = (head h, row-half off in {0, 2048}).  The "off"
cores compute in rolled coordinates (ft and bias rolled by -off along
both N axes on the host), so a single SPMD program computes rows
[0, 2048) for every core; the host un-rolls the output columns.

Per-core schedule (j split into two phases of 16 strips so the fp16 C
half [16 MB] stays SBUF-resident):
  phase A: elementwise C strips 0..15, column sums, G_partial = C_A^T C_A
           spilled to DRAM scratch.
  phase B: elementwise C strips 16..31, sums, total reciprocal, then
           t = (G_partial + C_B^T C_B) * r_i * r_k streamed to out.
"""

import sys

import numpy as np

sys.path.insert(0, "/opt/trn_rl_repo")

import concourse.bass as bass
import concourse.bacc as bacc
import concourse.mybir as mybir
import concourse.tile as tile
from concourse.bass_utils import run_bass_kernel_spmd
from concourse.tile_rust import add_dep_helper

F32 = mybir.dt.float32
F16 = mybir.dt.float16
AF = mybir.ActivationFunctionType

N = 4096
FT = 512
H = 4
NCORES = 8
SHIFT = 2.7725887  # 4*ln(2): keeps exp() comfortably inside fp16 range
ALPHA = 0.2
# The HW Lrelu LUT has a fixed negative slope SIGMA=0.01 (alpha is ignored).
# lrelu_0.2(z) == LA*f(z) + LB*f(-z) for f = lrelu_sigma: solve
#   a - sigma*b = 1,  sigma*a - b = 0.2
SIGMA = 0.01
LB = (ALPHA - SIGMA) / (SIGMA * SIGMA - 1.0)
LA = 1.0 + SIGMA * LB
ALU = mybir.AluOpType


def build_kernel(n=N, ft_dim=FT, upto=99):
    ns = n // 128            # total j strips
    nh = ns // 2             # strips per phase
    half = n // 2            # output rows per core
    qw = min(1024, n)        # elementwise sub-tile width
    kw = min(1024, n)        # matmul psum tile k-width (2 PSUM banks)
    cw = min(512, n)         # row-chunk width for [1, *] psum evictions
    nf = ft_dim // 128

    nc = bacc.Bacc(None, target_bir_lowering=False, debug=False)
    ftr = nc.dram_tensor("ftr", [ft_dim, n], F32, kind="ExternalInput")
    biasT = nc.dram_tensor("biasT", [n, n], F32, kind="ExternalInput")
    wh = nc.dram_tensor("wh", [1, ft_dim], F32, kind="ExternalInput")
    bh = nc.dram_tensor("bh", [1, 1], F32, kind="ExternalInput")
    out = nc.dram_tensor("out", [half, n], F32, kind="ExternalOutput")
    w_scr = nc.dram_tensor("w_scr", [1, n], F32)
    sa_scr = nc.dram_tensor("sa_scr", [1, n], F32)
    sb_scr = nc.dram_tensor("sb_scr", [1, n], F32)
    r_scr = nc.dram_tensor("r_scr", [1, n], F32)
    g_scr = nc.dram_tensor("g_scr", [half, n], F32)

    with tile.TileContext(nc) as tc:
        with tc.tile_pool(name="persist", bufs=1) as P, \
                tc.tile_pool(name="work", bufs=1) as WK, \
                tc.tile_pool(name="pwork", bufs=1, space="PSUM") as PW:
            ones_c = P.tile([128, 1], F16, tag="ones_c")
            negshift = P.tile([128, 1], F32, tag="negshift")
            whT = P.tile([128, nf], F32, tag="whT")
            bhs = P.tile([1, 1], F32, tag="bhs")
            wrow = P.tile([128, n], F32, tag="wrow")
            wcols = P.tile([128, ns], F32, tag="wcols")
            negwcols = P.tile([128, ns], F32, tag="negwcols")
            rcols = P.tile([128, ns], F32, tag="rcols")
            scolsa = P.tile([128, ns], F32, tag="scolsa")
            scolsb = P.tile([128, ns], F32, tag="scolsb")
            rbc = P.tile([128, n], F32, tag="rbc")
            C = [P.tile([128, n], F16, tag=f"C{s}", name=f"C{s}")
                 for s in range(nh)]

            nc.vector.memset(ones_c[:], 1.0)
            nc.vector.memset(negshift[:], -SHIFT)
            nc.sync.dma_start(whT[:], wh[0, :].rearrange("(f p) -> p f", p=128))
            nc.sync.dma_start(bhs[:], bh[:, :])

            # ---- w row: w = W[h] @ ft + b[h], evicted chunk-wise to DRAM ----
            w_stores = []
            for q in range(n // kw):
                fq = []
                for f in range(nf):
                    ftile = WK.tile([128, kw], F32, tag="ev", bufs=4, name=f"ft{q}_{f}")
                    nc.sync.dma_start(
                        ftile[:], ftr[f * 128:(f + 1) * 128, q * kw:(q + 1) * kw])
                    fq.append(ftile)
                for c in range(kw // cw):
                    psw = PW.tile([1, cw], F32, tag="prow", bufs=2, name=f"psw{q}_{c}")
                    for f in range(nf):
                        nc.tensor.matmul(
                            psw[:], whT[:, f:f + 1],
                            fq[f][:, c * cw:(c + 1) * cw],
                            start=(f == 0), stop=(f == nf - 1))
                    wev = WK.tile([1, cw], F32, tag="wev", bufs=2, name=f"wev{q}_{c}")
                    nc.scalar.activation(wev[:], psw[:], AF.Identity, bias=bhs[:])
                    w_stores.append(nc.sync.dma_start(
                        w_scr[0:1, q * kw + c * cw:q * kw + (c + 1) * cw], wev[:]))

            ld_wc = nc.sync.dma_start(
                wcols[:], w_scr[0, :].rearrange("(t p) -> p t", p=128))
            ld_wr = nc.sync.dma_start(
                wrow[:], w_scr[0, :][None, :].to_broadcast((128, n)))
            for st in w_stores:
                add_dep_helper(ld_wc.ins, st.ins, reason="w_scr RAW cols")
                add_dep_helper(ld_wr.ins, st.ins, reason="w_scr RAW row")
            nc.vector.tensor_scalar_mul(negwcols[:], wcols[:], -1.0)

            g_store = {}
            s_stores = [[], []]

            for ph in range(2 if upto >= 2 else 0):
                s_scr = sa_scr if ph == 0 else sb_scr
                # ---- elementwise: C[j, i] strips for this j half ----
                for s in range(nh):
                    js = ph * nh + s
                    for q in range(n // qw):
                        lo = q * qw
                        bt = WK.tile([128, qw], F32, tag="bt",
                                     bufs=2, name=f"bt{ph}_{s}_{q}")
                        nc.sync.dma_start(
                            bt[:], biasT[js * 128:(js + 1) * 128, lo:lo + qw])
                        sc = WK.tile([128, qw], F32, tag="sc",
                                     bufs=2, name=f"sc{ph}_{s}_{q}")
                        nc.scalar.activation(
                            sc[:], wrow[:, lo:lo + qw], AF.Lrelu,
                            bias=wcols[:, js:js + 1])
                        sc2 = WK.tile([128, qw], F32, tag="sc2",
                                      bufs=2, name=f"sc2{ph}_{s}_{q}")
                        nc.scalar.activation(
                            sc2[:], wrow[:, lo:lo + qw], AF.Lrelu,
                            bias=negwcols[:, js:js + 1], scale=-1.0)
                        nc.vector.scalar_tensor_tensor(
                            sc2[:], sc2[:], LB, bt[:], op0=ALU.mult, op1=ALU.add)
                        nc.vector.scalar_tensor_tensor(
                            sc[:], sc[:], LA, sc2[:], op0=ALU.mult, op1=ALU.add)
                        nc.scalar.activation(
                            C[s][:, lo:lo + qw], sc[:], AF.Exp,
                            bias=negshift[:])

                # ---- column sums of this half (PE contracts partitions) ----
                for c in range(n // cw if upto >= 3 else 0):
                    pss = PW.tile([1, cw], F32, tag="prow", bufs=2, name=f"pss{ph}_{c}")
                    for s in range(nh):
                        nc.tensor.matmul(
                            pss[:], ones_c[:], C[s][:, c * cw:(c + 1) * cw],
                            start=(s == 0), stop=(s == nh - 1))
                    sev = WK.tile([1, cw], F32, tag="wev", bufs=2,
                                   name=f"sev{ph}_{c}")
                    nc.scalar.copy(sev[:], pss[:])
                    s_stores[ph].append(nc.sync.dma_start(
                        s_scr[0:1, c * cw:(c + 1) * cw], sev[:]))

                if ph == 1 and upto >= 4:
                    # total sums -> reciprocals (columns), r broadcast (rows)
                    ld_sa = nc.sync.dma_start(
                        scolsa[:], sa_scr[0, :].rearrange("(t p) -> p t", p=128))
                    ld_sb = nc.sync.dma_start(
                        scolsb[:], sb_scr[0, :].rearrange("(t p) -> p t", p=128))
                    for st in s_stores[0]:
                        add_dep_helper(ld_sa.ins, st.ins, reason="sa RAW")
                    for st in s_stores[1]:
                        add_dep_helper(ld_sb.ins, st.ins, reason="sb RAW")
                    nc.vector.tensor_add(scolsa[:], scolsa[:], scolsb[:])
                    nc.vector.reciprocal(rcols[:], scolsa[:])
                    st_r = nc.sync.dma_start(
                        r_scr[0, :].rearrange("(t p) -> p t", p=128), rcols[:])
                    ld_rb = nc.sync.dma_start(
                        rbc[:], r_scr[0, :][None, :].to_broadcast((128, n)))
                    add_dep_helper(ld_rb.ins, st_r.ins, reason="r_scr RAW")

                # ---- G accumulation: ph0 spills partials, ph1 finalizes ----
                for ib in range(half // 128 if upto >= 5 else 0):
                    for kh in range(n // kw):
                        ps = PW.tile([128, kw], F32, tag="ps",
                                     bufs=3, name=f"ps{ph}_{ib}_{kh}")
                        for s in range(nh):
                            for cc in range(kw // 512):
                                col = kh * kw + cc * 512
                                nc.tensor.matmul(
                                    ps[:, cc * 512:(cc + 1) * 512],
                                    C[s][:, ib * 128:(ib + 1) * 128],
                                    C[s][:, col:col + 512],
                                    start=(s == 0), stop=(s == nh - 1))
                        if ph == 0:
                            ev = WK.tile([128, kw], F32, tag="ev",
                                         bufs=6, name=f"ev{ib}_{kh}")
                            nc.scalar.copy(ev[:], ps[:])
                            g_store[(ib, kh)] = nc.sync.dma_start(
                                g_scr[ib * 128:(ib + 1) * 128,
                                      kh * kw:(kh + 1) * kw], ev[:])
                        else:
                            ga = WK.tile([128, kw], F32, tag="ev",
                                         bufs=6, name=f"ga{ib}_{kh}")
                            ld = nc.sync.dma_start(
                                ga[:], g_scr[ib * 128:(ib + 1) * 128,
                                             kh * kw:(kh + 1) * kw])
                            add_dep_helper(
                                ld.ins, g_store[(ib, kh)].ins, reason="g RAW")
                            nc.vector.tensor_add(ga[:], ps[:], ga[:])
                            nc.scalar.activation(
                                ga[:], ga[:], AF.Copy,
                                scale=rcols[:, ib:ib + 1])
                            nc.vector.tensor_mul(
                                ga[:], ga[:], rbc[:, kh * kw:(kh + 1) * kw])
                            nc.sync.dma_start(
                                out[ib * 128:(ib + 1) * 128,
                                    kh * kw:(kh + 1) * kw], ga[:])
    return nc


def make_core_inputs(ft_mat, bias_mat, W, b, n=N):
    """Host-side shard prep: one input map per core (head, roll-offset)."""
    half = n // 2
    ft0 = np.asarray(ft_mat, dtype=np.float32)[0]
    bias0 = np.asarray(bias_mat, dtype=np.float32)[0]
    biasT0 = np.ascontiguousarray(bias0.T)
    ins = []
    for core in range(NCORES):
        h = core % H
        off = (core // H) * half
        ftr = np.ascontiguousarray(np.roll(ft0, -off, axis=1))
        if off:
            bT = np.ascontiguousarray(np.roll(biasT0, -off, axis=(0, 1)))
        else:
            bT = biasT0
        ins.append({
            "ftr": ftr,
            "biasT": bT,
            "wh": np.ascontiguousarray(np.asarray(W, np.float32)[h]).reshape(1, -1),
            "bh": np.asarray(b, np.float32)[h].reshape(1, 1),
        })
    return ins


def assemble_output(results, n=N):
    half = n // 2
    full = np.zeros((1, n, H * n), dtype=np.float32)
    for core in range(NCORES):
        h = core % H
        off = (core // H) * half
        o = results[core]["out"]
        full[0, off:off + half, h * n:(h + 1) * n] = np.roll(o, off, axis=1)
    return full


_nc_cache = {}
USE_V2 = False
USE_V3 = True


def build_active():
    if USE_V3:
        return build_kernel_v3()
    return build_kernel_v2() if USE_V2 else build_kernel()


def make_inputs_active(ft_mat, bias_mat, W, b):
    if USE_V3:
        return make_core_inputs_v3(ft_mat, bias_mat, W, b)
    if USE_V2:
        return make_core_inputs_v2(ft_mat, bias_mat, W, b)
    return make_core_inputs(ft_mat, bias_mat, W, b)


def assemble_active(results):
    if USE_V3:
        return assemble_output_v3(results)
    return (assemble_output_v2 if USE_V2 else assemble_output)(results)


def kernel(ft_mat, bias_mat, W, b):
    key = (N, USE_V2, USE_V3)
    if key not in _nc_cache:
        nc = build_active()
        nc.finalize()
        _nc_cache[key] = nc
    nc = _nc_cache[key]
    ins = make_inputs_active(ft_mat, bias_mat, W, b)
    res = run_bass_kernel_spmd(nc, ins, list(range(NCORES)))
    return assemble_active(res.results)


def build_kernel_v2(n=N, ft_dim=FT):
    """V2: exploit symmetry of t = C^T C.  Each core computes cyclic
    diagonals d in [0, nb/2] of the block grid for rows ib in [0, nb/2)
    and mirrors d in [1, nb/2) by PE-transposing finalized blocks.  The
    roll-offset core covers rows [nb/2, nb) of the same diagonal set, so
    together they tile the full symmetric matrix with ~half the MACs."""
    ns = n // 128
    nh = ns // 2
    half = n // 2
    nb = n // 128
    nbh = nb // 2            # ib range and also dmax
    dmax = nb // 2
    ngrp = (dmax + 3) // 4   # diagonal groups of up to 4 blocks (512 cols)
    qw = min(1024, n)
    cw = min(512, n)
    nf = ft_dim // 128
    gcols = dmax * 128 + 128

    nc = bacc.Bacc(None, target_bir_lowering=False, debug=False)
    ftr = nc.dram_tensor("ftr", [ft_dim, n], F32, kind="ExternalInput")
    biasT = nc.dram_tensor("biasT", [n, n], F32, kind="ExternalInput")
    wh = nc.dram_tensor("wh", [1, ft_dim], F32, kind="ExternalInput")
    bh = nc.dram_tensor("bh", [1, 1], F32, kind="ExternalInput")
    ident_d = nc.dram_tensor("ident", [128, 128], F32, kind="ExternalInput")
    out = nc.dram_tensor("out", [n, n], F32, kind="ExternalOutput")
    w_scr = nc.dram_tensor("w_scr", [1, n], F32)
    sa_scr = nc.dram_tensor("sa_scr", [1, n], F32)
    sb_scr = nc.dram_tensor("sb_scr", [1, n], F32)
    r_scr = nc.dram_tensor("r_scr", [1, n], F32)
    g_scr = nc.dram_tensor("g_scr", [half, gcols], F32)

    with tile.TileContext(nc) as tc:
        with tc.tile_pool(name="persist", bufs=1) as P, \
                tc.tile_pool(name="work", bufs=1) as WK, \
                tc.tile_pool(name="pwork", bufs=1, space="PSUM") as PW:
            ones_c = P.tile([128, 1], F16, tag="ones_c")
            negshift = P.tile([128, 1], F32, tag="negshift")
            whT = P.tile([128, nf], F32, tag="whT")
            bhs = P.tile([1, 1], F32, tag="bhs")
            ident = P.tile([128, 128], F32, tag="ident")
            wrow = P.tile([128, n], F32, tag="wrow")
            wcols = P.tile([128, ns], F32, tag="wcols")
            negwcols = P.tile([128, ns], F32, tag="negwcols")
            rcols = P.tile([128, ns], F32, tag="rcols")
            scolsa = P.tile([128, ns], F32, tag="scolsa")
            scolsb = P.tile([128, ns], F32, tag="scolsb")
            rbc = P.tile([128, n], F32, tag="rbc")
            C = [P.tile([128, n], F16, tag=f"C{s}", name=f"C{s}")
                 for s in range(nh)]

            nc.vector.memset(ones_c[:], 1.0)
            nc.vector.memset(negshift[:], -SHIFT)
            nc.sync.dma_start(whT[:], wh[0, :].rearrange("(f p) -> p f", p=128))
            nc.sync.dma_start(bhs[:], bh[:, :])
            nc.sync.dma_start(ident[:], ident_d[:, :])

            # ---- w row ----
            w_stores = []
            for q in range(n // qw):
                fq = []
                for f in range(nf):
                    ftile = WK.tile([128, qw], F32, tag="bt", bufs=2,
                                    name=f"ft{q}_{f}")
                    nc.sync.dma_start(
                        ftile[:], ftr[f * 128:(f + 1) * 128, q * qw:(q + 1) * qw])
                    fq.append(ftile)
                for c in range(qw // cw):
                    psw = PW.tile([1, cw], F32, tag="ps", bufs=6,
                                  name=f"psw{q}_{c}")
                    for f in range(nf):
                        nc.tensor.matmul(
                            psw[:], whT[:, f:f + 1],
                            fq[f][:, c * cw:(c + 1) * cw],
                            start=(f == 0), stop=(f == nf - 1))
                    wev = WK.tile([1, cw], F32, tag="wev", bufs=2,
                                  name=f"wev{q}_{c}")
                    nc.scalar.activation(wev[:], psw[:], AF.Identity, bias=bhs[:])
                    w_stores.append(nc.sync.dma_start(
                        w_scr[0:1, q * qw + c * cw:q * qw + (c + 1) * cw], wev[:]))

            ld_wc = nc.sync.dma_start(
                wcols[:], w_scr[0, :].rearrange("(t p) -> p t", p=128))
            ld_wr = nc.sync.dma_start(
                wrow[:], w_scr[0, :][None, :].to_broadcast((128, n)))
            for st in w_stores:
                add_dep_helper(ld_wc.ins, st.ins, reason="w RAW cols")
                add_dep_helper(ld_wr.ins, st.ins, reason="w RAW row")
            nc.vector.tensor_scalar_mul(negwcols[:], wcols[:], -1.0)

            g_store = {}
            s_stores = [[], []]

            for ph in range(2):
                s_scr = sa_scr if ph == 0 else sb_scr
                # ---- elementwise strips ----
                for s in range(nh):
                    js = ph * nh + s
                    for q in range(n // qw):
                        lo = q * qw
                        bt = WK.tile([128, qw], F32, tag="bt",
                                     bufs=2, name=f"bt{ph}_{s}_{q}")
                        nc.sync.dma_start(
                            bt[:], biasT[js * 128:(js + 1) * 128, lo:lo + qw])
                        sc = WK.tile([128, qw], F32, tag="sc",
                                     bufs=2, name=f"sc{ph}_{s}_{q}")
                        nc.scalar.activation(
                            sc[:], wrow[:, lo:lo + qw], AF.Lrelu,
                            bias=wcols[:, js:js + 1])
                        sc2 = WK.tile([128, qw], F32, tag="sc2",
                                      bufs=2, name=f"sc2{ph}_{s}_{q}")
                        nc.scalar.activation(
                            sc2[:], wrow[:, lo:lo + qw], AF.Lrelu,
                            bias=negwcols[:, js:js + 1], scale=-1.0)
                        nc.vector.scalar_tensor_tensor(
                            sc2[:], sc2[:], LB, bt[:], op0=ALU.mult, op1=ALU.add)
                        nc.vector.scalar_tensor_tensor(
                            sc[:], sc[:], LA, sc2[:], op0=ALU.mult, op1=ALU.add)
                        nc.scalar.activation(
                            C[s][:, lo:lo + qw], sc[:], AF.Exp,
                            bias=negshift[:])

                # ---- column sums ----
                for c in range(n // cw):
                    pss = PW.tile([1, cw], F32, tag="ps", bufs=6,
                                  name=f"pss{ph}_{c}")
                    for s in range(nh):
                        nc.tensor.matmul(
                            pss[:], ones_c[:], C[s][:, c * cw:(c + 1) * cw],
                            start=(s == 0), stop=(s == nh - 1))
                    sev = WK.tile([1, cw], F32, tag="wev", bufs=2,
                                  name=f"sev{ph}_{c}")
                    nc.scalar.copy(sev[:], pss[:])
                    s_stores[ph].append(nc.sync.dma_start(
                        s_scr[0:1, c * cw:(c + 1) * cw], sev[:]))

                if ph == 1:
                    ld_sa = nc.sync.dma_start(
                        scolsa[:], sa_scr[0, :].rearrange("(t p) -> p t", p=128))
                    ld_sb = nc.sync.dma_start(
                        scolsb[:], sb_scr[0, :].rearrange("(t p) -> p t", p=128))
                    for st in s_stores[0]:
                        add_dep_helper(ld_sa.ins, st.ins, reason="sa RAW")
                    for st in s_stores[1]:
                        add_dep_helper(ld_sb.ins, st.ins, reason="sb RAW")
                    nc.vector.tensor_add(scolsa[:], scolsa[:], scolsb[:])
                    nc.vector.reciprocal(rcols[:], scolsa[:])
                    st_r = nc.sync.dma_start(
                        r_scr[0, :].rearrange("(t p) -> p t", p=128), rcols[:])
                    ld_rb = nc.sync.dma_start(
                        rbc[:], r_scr[0, :][None, :].to_broadcast((128, n)))
                    add_dep_helper(ld_rb.ins, st_r.ins, reason="r RAW")

                # ---- symmetric G: diagonals d in [0, dmax), grouped ----
                for ib in range(nbh):
                    tiles = []
                    gws = []
                    for dg in range(ngrp):
                        gwd = min(4, dmax - dg * 4) * 128
                        gws.append(gwd)
                        tiles.append(PW.tile([128, gwd], F32, tag="ps", bufs=6,
                                             name=f"ps{ph}_{ib}_{dg}"))
                    for s in range(nh):
                        lhsT = C[s][:, ib * 128:(ib + 1) * 128]
                        for dg in range(ngrp):
                            c0 = (ib * 128 + dg * 512) % n
                            w1 = min(gws[dg], n - c0)
                            nc.tensor.matmul(
                                tiles[dg][:, 0:w1], lhsT, C[s][:, c0:c0 + w1],
                                start=(s == 0), stop=(s == nh - 1))
                            if w1 < gws[dg]:
                                nc.tensor.matmul(
                                    tiles[dg][:, w1:gws[dg]], lhsT,
                                    C[s][:, 0:gws[dg] - w1],
                                    start=(s == 0), stop=(s == nh - 1))
                    for dg in range(ngrp):
                        gwd = gws[dg]
                        c0 = (ib * 128 + dg * 512) % n
                        w1 = min(gwd, n - c0)
                        gc = dg * 512
                        if ph == 0:
                            ev = WK.tile([128, gwd], F32, tag="ev", bufs=6,
                                         name=f"ev{ib}_{dg}")
                            nc.scalar.copy(ev[:], tiles[dg][:])
                            g_store[(ib, dg)] = nc.sync.dma_start(
                                g_scr[ib * 128:(ib + 1) * 128, gc:gc + gwd],
                                ev[:])
                        else:
                            ga = WK.tile([128, gwd], F32, tag="ev", bufs=6,
                                         name=f"ga{ib}_{dg}")
                            ld = nc.sync.dma_start(
                                ga[:], g_scr[ib * 128:(ib + 1) * 128,
                                             gc:gc + gwd])
                            add_dep_helper(ld.ins, g_store[(ib, dg)].ins,
                                           reason="g RAW")
                            nc.vector.tensor_add(ga[:], tiles[dg][:], ga[:])
                            nc.scalar.activation(
                                ga[:], ga[:], AF.Copy,
                                scale=rcols[:, ib:ib + 1])
                            nc.vector.tensor_mul(
                                ga[:, 0:w1], ga[:, 0:w1], rbc[:, c0:c0 + w1])
                            nc.sync.dma_start(
                                out[ib * 128:(ib + 1) * 128, c0:c0 + w1],
                                ga[:, 0:w1])
                            if w1 < gwd:
                                nc.vector.tensor_mul(
                                    ga[:, w1:gwd], ga[:, w1:gwd],
                                    rbc[:, 0:gwd - w1])
                                nc.sync.dma_start(
                                    out[ib * 128:(ib + 1) * 128, 0:gwd - w1],
                                    ga[:, w1:gwd])
                            for dd in range(gwd // 128):
                                d = dg * 4 + dd
                                if d == 0:
                                    continue
                                kb = (ib + d) % nb
                                pt = PW.tile([128, 128], F32, tag="pt", bufs=2,
                                             name=f"pt{ib}_{d}")
                                nc.tensor.transpose(
                                    pt[:], ga[:, dd * 128:(dd + 1) * 128],
                                    ident[:])
                                mv = WK.tile([128, 128], F32, tag="mv", bufs=3,
                                             name=f"mv{ib}_{d}")
                                nc.scalar.copy(mv[:], pt[:])
                                nc.sync.dma_start(
                                    out[kb * 128:(kb + 1) * 128,
                                        ib * 128:(ib + 1) * 128], mv[:])

                # ---- d = dmax pass (self-paired diagonal, direct only) ----
                for ib in range(nbh):
                    kb = ib + dmax
                    pd = PW.tile([128, 128], F32, tag="ps", bufs=6,
                                 name=f"pd{ph}_{ib}")
                    for s in range(nh):
                        nc.tensor.matmul(
                            pd[:], C[s][:, ib * 128:(ib + 1) * 128],
                            C[s][:, kb * 128:(kb + 1) * 128],
                            start=(s == 0), stop=(s == nh - 1))
                    if ph == 0:
                        ev = WK.tile([128, 128], F32, tag="ev", bufs=6,
                                     name=f"evd{ib}")
                        nc.scalar.copy(ev[:], pd[:])
                        g_store[(ib, "dmax")] = nc.sync.dma_start(
                            g_scr[ib * 128:(ib + 1) * 128,
                                  dmax * 128:dmax * 128 + 128], ev[:])
                    else:
                        ga = WK.tile([128, 128], F32, tag="ev", bufs=6,
                                     name=f"gad{ib}")
                        ld = nc.sync.dma_start(
                            ga[:], g_scr[ib * 128:(ib + 1) * 128,
                                         dmax * 128:dmax * 128 + 128])
                        add_dep_helper(ld.ins, g_store[(ib, "dmax")].ins,
                                       reason="gd RAW")
                        nc.vector.tensor_add(ga[:], pd[:], ga[:])
                        nc.scalar.activation(
                            ga[:], ga[:], AF.Copy, scale=rcols[:, ib:ib + 1])
                        nc.vector.tensor_mul(
                            ga[:], ga[:], rbc[:, kb * 128:(kb + 1) * 128])
                        nc.sync.dma_start(
                            out[ib * 128:(ib + 1) * 128,
                                kb * 128:(kb + 1) * 128], ga[:])
    return nc


SHIFT3 = 4.0            # global exp shift for v3 (fp16 C head-room)
DMAX = 16               # cyclic block diagonals covered per core: d in [0, 16]
BAND = (DMAX + 1) * 128  # 2176 compact band columns


def build_kernel_v3(n=N, ft_dim=FT, g5b=6, g1b=1, prb=1, kdve=0, wkb=5):
    """V3: fp16 end-to-end data path, PE-bound schedule.

    Per core (head h, row-half off): C[j, i] = exp(0.4|z| + 0.6 w_i +
    0.6 w_j + biasT[j, i] - SHIFT3), z = w_i + w_j, built in two 16-strip
    phases.  Per phase, the symmetric band G[ib, ib..ib+16] accumulates in
    PSUM over the resident strips (phase A spills raw fp32 partials to
    DRAM, phase B adds them back and normalizes).  The device emits only
    the compact band out[2048, 2176] in fp16; the host mirrors transposed
    blocks into the full matrix (t is symmetric), so no PE transposes,
    no mirror DMAs.
    """
    ns = n // 128
    nh = ns // 2
    half = n // 2
    nbh = ns // 2
    nf = ft_dim // 128
    cw = 512

    nc = bacc.Bacc(None, target_bir_lowering=False, debug=False)
    biasT16 = nc.dram_tensor("biasT16", [n, n], F16, kind="ExternalInput")
    ftr16 = nc.dram_tensor("ftr16", [ft_dim, n], F16, kind="ExternalInput")
    wh16 = nc.dram_tensor("wh16", [1, ft_dim], F16, kind="ExternalInput")
    bh = nc.dram_tensor("bh", [1, 1], F32, kind="ExternalInput")
    outc = nc.dram_tensor("outc", [half, BAND], F16, kind="ExternalOutput")
    w_scr = nc.dram_tensor("w_scr", [1, n], F32)
    w16_scr = nc.dram_tensor("w16_scr", [1, n], F16)
    sa_scr = nc.dram_tensor("sa_scr", [1, n], F32)
    sb_scr = nc.dram_tensor("sb_scr", [1, n], F32)
    r16_scr = nc.dram_tensor("r16_scr", [1, n], F16)
    g_scr = nc.dram_tensor("g_scr", [half, BAND], F32)

    with tile.TileContext(nc) as tc:
        with tc.tile_pool(name="persist", bufs=1) as P, \
                tc.tile_pool(name="work", bufs=1) as WK, \
                tc.tile_pool(name="pwork", bufs=1, space="PSUM") as PW:
            ones16 = P.tile([128, 1], F16, tag="ones16")
            whT = P.tile([128, nf], F16, tag="whT")
            bhs = P.tile([1, 1], F32, tag="bhs")
            wcols = P.tile([128, ns], F32, tag="wcols")
            wcols04 = P.tile([128, ns], F32, tag="wcols04")
            vcols06 = P.tile([128, ns], F32, tag="vcols06")
            rcols = P.tile([128, ns], F32, tag="rcols")
            rcols16 = P.tile([128, ns], F16, tag="rcols16")
            scols = P.tile([128, ns], F32, tag="scols")
            scolsb = P.tile([128, ns], F32, tag="scolsb")
            wrow16 = P.tile([128, n], F16, tag="wrow16")
            urow = P.tile([128, n], F16, tag="urow")
            rbc16 = P.tile([128, n], F16, tag="rbc16")
            C = [P.tile([128, n], F16, tag=f"C{s}", name=f"C{s}")
                 for s in range(nh)]

            nc.vector.memset(ones16[:], 1.0)
            nc.sync.dma_start(whT[:], wh16[0, :].rearrange("(f p) -> p f", p=128))
            nc.sync.dma_start(bhs[:], bh[:, :])

            # ---- w row: w = W[h] @ ft + b[h] (fp16 inputs, fp32 psum) ----
            w_stores, w16_stores = [], []
            fq = []
            for f in range(nf):
                ftile = WK.tile([128, n], F16, tag="wk", bufs=wkb, name=f"ft{f}")
                nc.sync.dma_start(ftile[:], ftr16[f * 128:(f + 1) * 128, :])
                fq.append(ftile)
            for c in range(n // cw):
                psw = PW.tile([1, cw], F32, tag="pr", bufs=prb, name=f"psw{c}")
                for f in range(nf):
                    nc.tensor.matmul(
                        psw[:], whT[:, f:f + 1], fq[f][:, c * cw:(c + 1) * cw],
                        start=(f == 0), stop=(f == nf - 1))
                wev = WK.tile([1, cw], F32, tag="sm", bufs=4, name=f"wev{c}")
                nc.scalar.activation(wev[:], psw[:], AF.Identity, bias=bhs[:])
                w_stores.append(nc.sync.dma_start(
                    w_scr[0:1, c * cw:(c + 1) * cw], wev[:]))
                wev16 = WK.tile([1, cw], F16, tag="sm", bufs=4, name=f"wev16{c}")
                nc.vector.tensor_scalar(wev16[:], wev[:], 1.0, None, op0=ALU.mult)
                w16_stores.append(nc.sync.dma_start(
                    w16_scr[0:1, c * cw:(c + 1) * cw], wev16[:]))

            ld_wc = nc.sync.dma_start(
                wcols[:], w_scr[0, :].rearrange("(t p) -> p t", p=128))
            for st in w_stores:
                add_dep_helper(ld_wc.ins, st.ins, reason="w RAW cols")
            ld_wr = nc.sync.dma_start(
                wrow16[:], w16_scr[0, :][None, :].to_broadcast((128, n)))
            for st in w16_stores:
                add_dep_helper(ld_wr.ins, st.ins, reason="w16 RAW row")
            nc.vector.tensor_scalar(wcols04[:], wcols[:], 0.4, None, op0=ALU.mult)
            nc.vector.tensor_scalar(vcols06[:], wcols[:], 0.6, None, op0=ALU.mult)
            nc.vector.tensor_scalar(urow[:], wrow16[:], 0.6, -SHIFT3,
                                    op0=ALU.mult, op1=ALU.add)
            w04row = None
            if kdve:
                w04row = P.tile([128, n], F16, tag="w04row")
                nc.vector.tensor_scalar(w04row[:], wrow16[:], 0.4, None,
                                        op0=ALU.mult)
            dve_set = set(np.linspace(0, 15, kdve, dtype=int)) if kdve else set()

            s_stores = [[], []]
            g_store = {}

            for ph in range(2):
                s_scr = sa_scr if ph == 0 else sb_scr
                # ---- elementwise strips (2 Act + 2 DVE ops per strip) ----
                for s in range(nh):
                    js = ph * nh + s
                    bt = WK.tile([128, n], F16, tag="wk", bufs=wkb,
                                 name=f"bt{ph}_{s}")
                    nc.sync.dma_start(bt[:], biasT16[js * 128:(js + 1) * 128, :])
                    nc.vector.tensor_add(bt[:], bt[:], urow[:])
                    av = WK.tile([128, n], F16, tag="wk", bufs=wkb,
                                 name=f"a{ph}_{s}")
                    if s in dve_set:
                        nc.vector.tensor_scalar(
                            av[:], w04row[:], wcols04[:, js:js + 1], 0.0,
                            op0=ALU.add, op1=ALU.abs_max)
                    else:
                        nc.scalar.activation(av[:], wrow16[:], AF.Abs,
                                             bias=wcols04[:, js:js + 1],
                                             scale=0.4)
                    nc.vector.tensor_add(av[:], av[:], bt[:])
                    nc.scalar.activation(C[s][:], av[:], AF.Exp,
                                         bias=vcols06[:, js:js + 1])

                # ---- column sums of this phase (fp16 ones matmul) ----
                for c in range(n // cw):
                    pss = PW.tile([1, cw], F32, tag="pr", bufs=prb,
                                  name=f"pss{ph}_{c}")
                    for s in range(nh):
                        nc.tensor.matmul(
                            pss[:], ones16[:], C[s][:, c * cw:(c + 1) * cw],
                            start=(s == 0), stop=(s == nh - 1))
                    sev = WK.tile([1, cw], F32, tag="sm", bufs=4,
                                  name=f"sev{ph}_{c}")
                    nc.scalar.copy(sev[:], pss[:])
                    s_stores[ph].append(nc.sync.dma_start(
                        s_scr[0:1, c * cw:(c + 1) * cw], sev[:]))

                if ph == 1:
                    ld_sa = nc.sync.dma_start(
                        scols[:], sa_scr[0, :].rearrange("(t p) -> p t", p=128))
                    ld_sb = nc.sync.dma_start(
                        scolsb[:], sb_scr[0, :].rearrange("(t p) -> p t", p=128))
                    for st in s_stores[0]:
                        add_dep_helper(ld_sa.ins, st.ins, reason="sa RAW")
                    for st in s_stores[1]:
                        add_dep_helper(ld_sb.ins, st.ins, reason="sb RAW")
                    nc.vector.tensor_add(scols[:], scols[:], scolsb[:])
                    nc.vector.reciprocal(rcols[:], scols[:])
                    nc.vector.tensor_scalar(rcols16[:], rcols[:], 1.0, None,
                                            op0=ALU.mult)
                    st_r16 = nc.sync.dma_start(
                        r16_scr[0, :].rearrange("(t p) -> p t", p=128),
                        rcols16[:])
                    ld_rb = nc.sync.dma_start(
                        rbc16[:], r16_scr[0, :][None, :].to_broadcast((128, n)))
                    add_dep_helper(ld_rb.ins, st_r16.ins, reason="r16 RAW")

                # ---- banded G sweep: diagonals d in [0, 16] per ib ----
                for ib in range(nbh):
                    base = ib * 128
                    tiles = []
                    for dg in range(4):
                        tiles.append(PW.tile([128, 512], F32, tag="g5", bufs=g5b,
                                             name=f"ps{ph}_{ib}_{dg}"))
                    pd = PW.tile([128, 128], F32, tag="g1", bufs=g1b,
                                 name=f"pd{ph}_{ib}")
                    for s in range(nh):
                        lhsT = C[s][:, base:base + 128]
                        for dg in range(4):
                            c0 = base + dg * 512
                            nc.tensor.matmul(
                                tiles[dg][:], lhsT, C[s][:, c0:c0 + 512],
                                start=(s == 0), stop=(s == nh - 1))
                        c0 = base + 2048
                        nc.tensor.matmul(
                            pd[:], lhsT, C[s][:, c0:c0 + 128],
                            start=(s == 0), stop=(s == nh - 1))
                    if ph == 0:
                        stage = WK.tile([128, BAND], F32, tag="wk", bufs=wkb,
                                        name=f"stg{ib}")
                        for dg in range(4):
                            nc.vector.tensor_scalar(
                                stage[:, dg * 512:(dg + 1) * 512],
                                tiles[dg][:], 1.0, None, op0=ALU.mult)
                        nc.vector.tensor_scalar(
                            stage[:, 2048:2176], pd[:], 1.0, None, op0=ALU.mult)
                        g_store[ib] = nc.sync.dma_start(
                            g_scr[base:base + 128, :], stage[:])
                    else:
                        stage = WK.tile([128, BAND], F32, tag="wk", bufs=wkb,
                                        name=f"stgb{ib}")
                        ld = nc.sync.dma_start(
                            stage[:], g_scr[base:base + 128, :])
                        add_dep_helper(ld.ins, g_store[ib].ins, reason="g RAW")
                        o16 = WK.tile([128, BAND], F16, tag="wk", bufs=wkb,
                                      name=f"o16{ib}")
                        for dg in range(4):
                            sl = slice(dg * 512, (dg + 1) * 512)
                            c0 = base + dg * 512
                            nc.vector.tensor_add(
                                stage[:, sl], stage[:, sl], tiles[dg][:])
                            nc.vector.scalar_tensor_tensor(
                                o16[:, sl], stage[:, sl], rcols[:, ib:ib + 1],
                                rbc16[:, c0:c0 + 512],
                                op0=ALU.mult, op1=ALU.mult)
                        sl = slice(2048, 2176)
                        c0 = base + 2048
                        nc.vector.tensor_add(stage[:, sl], stage[:, sl], pd[:])
                        nc.vector.scalar_tensor_tensor(
                            o16[:, sl], stage[:, sl], rcols[:, ib:ib + 1],
                            rbc16[:, c0:c0 + 128], op0=ALU.mult, op1=ALU.mult)
                        nc.sync.dma_start(outc[base:base + 128, :], o16[:])
    return nc


def make_core_inputs_v3(ft_mat, bias_mat, W, b, n=N):
    half = n // 2
    ft0 = np.asarray(ft_mat, dtype=np.float32)[0]
    bias0 = np.asarray(bias_mat, dtype=np.float32)[0]
    biasT0 = np.ascontiguousarray(bias0.T)
    ins = []
    for core in range(NCORES):
        h = core % H
        off = (core // H) * half
        ftr = np.roll(ft0, -off, axis=1) if off else ft0
        bT = np.roll(biasT0, -off, axis=(0, 1)) if off else biasT0
        ins.append({
            "ftr16": np.ascontiguousarray(ftr.astype(np.float16)),
            "biasT16": np.ascontiguousarray(bT.astype(np.float16)),
            "wh16": np.asarray(W, np.float16)[h].reshape(1, -1),
            "bh": np.asarray(b, np.float32)[h].reshape(1, 1),
        })
    return ins


def assemble_output_v3(results, n=N):
    nb = n // 128
    nbh = nb // 2
    full = np.zeros((1, n, H * n), np.float32)
    ibs = np.repeat(np.arange(nbh), DMAX + 1)
    ds = np.tile(np.arange(DMAX + 1), nbh)
    kA = (ibs + ds) % nb
    for h in range(H):
        blocks = np.zeros((nb, nb, 128, 128), np.float32)
        OA = np.asarray(results[h]["outc"], np.float32)
        OB = np.asarray(results[h + H]["outc"], np.float32)
        bA = OA.reshape(nbh, 128, DMAX + 1, 128).transpose(0, 2, 1, 3)
        bB = OB.reshape(nbh, 128, DMAX + 1, 128).transpose(0, 2, 1, 3)
        # mirrors first (transposed), direct blocks after (they win ties)
        blocks[kA, ibs] = bA[ibs, ds].transpose(0, 2, 1)
        blocks[(kA + nbh) % nb, (ibs + nbh) % nb] = bB[ibs, ds].transpose(0, 2, 1)
        blocks[(ibs + nbh) % nb, (kA + nbh) % nb] = bB[ibs, ds]
        blocks[ibs, kA] = bA[ibs, ds]
        full[0, :, h * n:(h + 1) * n] = (
            blocks.transpose(0, 2, 1, 3).reshape(n, n))
    return full


def make_core_inputs_v2(ft_mat, bias_mat, W, b, n=N):
    ins = make_core_inputs(ft_mat, bias_mat, W, b, n)
    eye = np.ascontiguousarray(np.eye(128, dtype=np.float32))
    for m in ins:
        m["ident"] = eye
    return ins


def block_mask_v2(n=N):
    nb = n // 128
    dmax = nb // 2
    maskA = np.zeros((nb, nb), bool)
    for ib in range(nb // 2):
        for d in range(dmax + 1):
            kb = (ib + d) % nb
            maskA[ib, kb] = True
            if 1 <= d <= dmax - 1:
                maskA[kb, ib] = True
    maskB = np.roll(maskA, (nb // 2, nb // 2), (0, 1))
    assert not (maskA & maskB).any() and (maskA | maskB).all(), \
        "v2 block split is not a disjoint cover"
    return maskA


def assemble_output_v2(results, n=N):
    maskA = block_mask_v2(n)
    mask_big = np.repeat(np.repeat(maskA, 128, 0), 128, 1)
    full = np.zeros((1, n, H * n), np.float32)
    for h in range(H):
        tA = results[h]["out"]
        tB = np.roll(results[h + H]["out"], n // 2, axis=(0, 1))
        full[0, :, h * n:(h + 1) * n] = np.where(mask_big, tA, tB)
    return full

